# revision 51
# baseline (speedup 1.0000x reference)
"""KANLayer (in=128, out=128, num=5, k=3, batch=1024) on 8 trn2 NeuronCores.

Math: out[b,o] = sum_i mask*scale_base*silu(x[b,i])
              + sum_i mask*scale_sp*sum_j coef[(o,i),j]*B_j(x[b,i])
The reference grid is a uniform linspace broadcast to all rows, so the
Cox-de-Boor bases are cardinal cubic B-splines, B_j(v) = Delta^4
relu(v-n)^3/6 at n=j with v = (x - g0ext)/h.  The Delta^4 is a fixed
linear map from truncated-power taps to bases, so it is folded into the
spline weights host-side: y_sp = sum_n W[s,n]*relu(v-n)^3 with
W = (coef/6) @ Delta4^T.  Three exact host-side rewrites then keep every
on-device lane value small enough for bf16 throughout:

  * taps with knot >= max(v) are identically zero -> dropped;
  * taps with knot <= min(v) satisfy relu(v-n)^3 = (v-n)^3 -> folded
    into a cubic polynomial;
  * live taps below the range midpoint use the mirror identity
    relu(v-n)^3 = (v-n)^3 + relu(n-v)^3, the cubic again folded into
    the polynomial.

Everything is evaluated in x-space (u = x - knot_x, the 1/h^k scales
folded into the weights).  The device-side work per pass is one bf16
lane bank [128, 7, 128] = {6 live knots, 1 poly-center lane} built by
four DVE ops (subtract; cube via square+multiply with the square on the
scalar engine; relu as min over mirrored lanes + max over direct lanes,
with the mirror sign folded into the weights since relu(knot-x)^3 =
-min(u^3, 0) and relu/cube commute), plus silu on the scalar engine.
The poly-center lane's powers in DD/U2/U3 are the polynomial features
w, w^2, w^3 for free, and the constant term rides the PSUM->SBUF output
copy as a per-partition bias.  The contraction is 10 accumulated
128x128x128 bf16 PE matmuls per core against a weight bank that stays
resident in SBUF.  All elementwise ops run in bf16 (2x/4x DVE modes).

Sharding: batch 1024 -> 128 per core (independent; no collectives).

Execution: the Bass program is AOT-compiled once into a PJRT executable
(fast-dispatch, no per-call retrace) and dispatched on cores 0-7; falls
back to the stock run_bass_kernel_spmd path on any failure.
"""

import numpy as np

import concourse.bass as bass
import concourse.mybir as mybir
import concourse.tile as tile

AF = mybir.ActivationFunctionType
ALU = mybir.AluOpType
F32 = mybir.dt.float32
BF16 = mybir.dt.bfloat16

N_CORES = 8
BATCH = 1024
IN_DIM = 128
OUT_DIM = 128
NUM, KDEG = 5, 3
NB = NUM + KDEG          # 8 basis functions
NT = NB + KDEG + 1       # 12 truncated-power taps
BSH = BATCH // N_CORES   # 128 batch elems per core
SIZE = IN_DIM * OUT_DIM

MM_DT = BF16  # matmul operand dtype


def _bcast_mid(ap2d, n):
    """[128, F] AP -> [128, n, F] with zero-stride middle dim."""
    p, f = ap2d.shape
    return ap2d.rearrange("p (a b) -> p a b", a=1).broadcast_to([p, n, f])


def _flat(ap3d):
    """[128, a, b] AP -> [128, a*b]."""
    return ap3d.rearrange("p a b -> p (a b)")


class Cfg:
    """Data-dependent program constants (live taps, split, poly center)."""

    def __init__(self, inv_h, bias_v, vmin, vmax):
        self.inv_h = float(inv_h)
        self.bias_v = float(bias_v)
        nlo = int(np.floor(vmin))          # taps <= nlo: always-on cubic
        nhi = int(np.ceil(vmax))           # taps >= nhi: identically zero
        self.cc = float((vmin + vmax) / 2)  # poly/tap rebase center
        self.msp = int(np.floor(self.cc))   # mirror split knot
        self.live = [n for n in range(max(nlo + 1, 0), min(nhi, NT))]
        self.nmir = sum(1 for n in self.live if n <= self.msp)
        self.ntap = len(self.live)
        self.nlanes = self.ntap + 1        # taps + the w-lane (t=0)
        self.ntiles = 4 + self.ntap        # silu, w, w^2, w^3, taps
        self.h = 1.0 / self.inv_h
        self.g0ext = -self.bias_v * self.h  # extended-grid origin (x-space)
        self.cx = self.g0ext + self.cc * self.h  # poly center, x-space
        # engine assignment knobs (chosen by timeline-sim + on-device sweep)
        # (GPSIMD cannot access PSUM, so o_eng is scalar or vector only)
        self.o_eng = "split"   # PSUM->SBUF output copy engine
        self.u2_act = True     # lane square on Act engine (else DVE)
        self.nbufs = 3         # passes in flight (tile buffer depth)
        self.out_pool = False  # issue the output DMA from the Pool queue
        self.skew = True       # emit pass k's output stage after pass k+1
        self.skew_depth = 1    # passes between compute and its output stage
        self.psum_extra = 1    # extra PSUM buffers beyond nbufs
        self.out_first = False  # emit pending output stage before compute
        # GpSimd per-op overhead is ~1us on real HW (cost model is wrong
        # about it) - never assign per-pass ops there
        self.poly_pool = False  # poly-feature lane pipeline on GpSimd
        self.o_dve_cols = 32   # columns of the output copy done on DVE
        self.dd2 = False       # two-op DD (mir lanes negated) + single max


def _emit_out(nc, pool, PS, outT, BV, cfg):
    """Output stage: PSUM -> SBUF copy (+ poly-constant bias), then DMA."""
    O = pool.tile([OUT_DIM, BSH], F32, tag="O", bufs=cfg.nbufs)
    if cfg.o_eng == "vector":
        nc.vector.tensor_scalar(O[:], PS[:], BV[:, 0:1], None, ALU.add)
    elif cfg.o_eng == "split":  # split the copy across DVE and Act
        h = cfg.o_dve_cols
        nc.vector.tensor_scalar(O[:, :h], PS[:, :h], BV[:, 0:1], None, ALU.add)
        nc.scalar.activation(O[:, h:], PS[:, h:], AF.Identity, bias=BV[:, 0:1])
    else:
        nc.scalar.activation(O[:], PS[:], AF.Identity, bias=BV[:, 0:1])
    (nc.gpsimd if cfg.out_pool else nc.sync).dma_start(outT[:, :], O[:])


def _emit_iter(nc, pool, psum, xs, WT, ICW, cfg):
    """One pass's compute: load, lane bank, 4+ntap matmuls -> PSUM tile."""
    ntap, nmir = cfg.ntap, cfg.nmir
    ib = cfg.nbufs  # intermediate-tile buffering (passes in flight)
    # multi-buffered input load (x arrives pre-cast to bf16): later
    # passes' DMAs issue while this pass computes (standard prefetch)
    XB = pool.tile([128, BSH], MM_DT, tag="XB", bufs=ib)
    nc.sync.dma_start(XB[:], xs[:])

    S = pool.tile([128, BSH], MM_DT, tag="S", bufs=ib)  # silu(x), K-tile 0
    nc.scalar.activation(S[:], XB[:], AF.Silu)

    # lane bank in x-space: u = x - knot_x per live knot, plus a final
    # lane at the poly center whose powers are the poly features (the
    # 1/h^k scalings are folded into the weights host-side).  relu(u)^3 =
    # relu(u^3), and the mirrored (below-split) knots need relu(knot-x)^3
    # = -min(u^3, 0), so the sign fold goes into their weights and the
    # relu stage is one min over mirrored lanes + one max over direct
    # lanes (4x-mode tensor_scalar).  All bf16 (2x/4x DVE modes).
    # with poly_pool the bank holds only the tap lanes; the poly-center
    # lane (no relu needed) runs as its own 3-op pipeline on GpSimd
    nb = ntap if cfg.poly_pool else cfg.nlanes
    DD = pool.tile([128, nb, BSH], MM_DT, tag="DD", bufs=ib)
    if cfg.dd2:
        # mirrored lanes hold knot-x directly, so the relu stage is one
        # max over all tap lanes and no weight sign fold is needed
        nc.vector.tensor_tensor(
            DD[:, :nmir, :], ICW[:, :nmir, :], _bcast_mid(XB[:], nmir),
            ALU.subtract,
        )
        nc.vector.tensor_tensor(
            DD[:, nmir:, :], _bcast_mid(XB[:], nb - nmir), ICW[:, nmir:nb, :],
            ALU.subtract,
        )
    else:
        nc.vector.tensor_tensor(
            DD[:], _bcast_mid(XB[:], nb), ICW[:, :nb, :], ALU.subtract
        )
    U2 = pool.tile([128, nb, BSH], MM_DT, tag="U2", bufs=ib)
    if cfg.u2_act:
        nc.scalar.activation(_flat(U2[:]), _flat(DD[:]), AF.Square)
    else:
        nc.vector.tensor_tensor(_flat(U2[:]), _flat(DD[:]), _flat(DD[:]),
                                ALU.mult)
    U3 = pool.tile([128, nb, BSH], MM_DT, tag="U3", bufs=ib)
    nc.vector.tensor_tensor(_flat(U3[:]), _flat(U2[:]), _flat(DD[:]), ALU.mult)
    R3 = pool.tile([128, ntap, BSH], MM_DT, tag="R3", bufs=ib)
    if cfg.dd2:
        nc.vector.tensor_scalar(
            _flat(R3[:]), _flat(U3[:])[:, : ntap * BSH], 0.0, None, ALU.max
        )
    else:
        nc.vector.tensor_scalar(
            _flat(R3[:])[:, : nmir * BSH], _flat(U3[:])[:, : nmir * BSH],
            0.0, None, ALU.min,
        )
        nc.vector.tensor_scalar(
            _flat(R3[:])[:, nmir * BSH :],
            _flat(U3[:])[:, nmir * BSH : ntap * BSH], 0.0, None, ALU.max,
        )
    if cfg.poly_pool:
        DP = pool.tile([128, BSH], MM_DT, tag="DP", bufs=ib)
        nc.gpsimd.tensor_scalar(DP[:], XB[:], cfg.cx, None, ALU.subtract)
        P2 = pool.tile([128, BSH], MM_DT, tag="P2", bufs=ib)
        nc.gpsimd.tensor_tensor(P2[:], DP[:], DP[:], ALU.mult)
        P3 = pool.tile([128, BSH], MM_DT, tag="P3", bufs=ib)
        nc.gpsimd.tensor_tensor(P3[:], P2[:], DP[:], ALU.mult)
        poly = [DP[:], P2[:], P3[:]]
    else:
        poly = [DD[:, ntap, :], U2[:, ntap, :], U3[:, ntap, :]]

    # out^T[o,b] = sum_k WT[:,k,:]^T @ rhs_k, K = ntiles*128
    PS = psum.tile([OUT_DIM, BSH], F32, tag="PS",
                   bufs=cfg.nbufs + (cfg.psum_extra if cfg.skew else 0))
    rhss = [S[:]] + poly
    rhss += [R3[:, t, :] for t in range(ntap)]
    for k, rhs in enumerate(rhss):
        nc.tensor.matmul(
            PS[:], WT[:, k, :], rhs, start=(k == 0), stop=(k == len(rhss) - 1)
        )
    return PS


def build_program(
    cfg, iters: int = 1, pipelined: bool = False, loop_n: int = 1
):
    """One SPMD NeuronCore program; per-core inputs differ only in data.

    iters > 1 unrolls the full kernel back-to-back inside one NEFF, and
    loop_n > 1 wraps the unrolled body in a hardware For_i loop (total
    passes = iters * loop_n) - used to measure per-iteration HW execution
    time without a profiler while keeping the NEFF small.

    Successive passes write a small ring of output slices (a real stream
    writes each batch's result to a distinct buffer; reusing one address
    would add an artificial DRAM write-after-write serialization to the
    measurement).  Slice 0 always holds a complete pass result.
    """
    del pipelined  # legacy knob, superseded by the output ring
    nc = bass.Bass()
    xs = nc.declare_dram_parameter("xs", [IN_DIM, BSH], MM_DT, isOutput=False)
    # weights pre-transposed host-side to [i, k*o] so the load is one
    # contiguous-per-partition DMA
    wt = nc.declare_dram_parameter(
        "wt", [128, cfg.ntiles * OUT_DIM], MM_DT, isOutput=False
    )
    icw = nc.declare_dram_parameter(
        "icw", [128, cfg.nlanes * BSH], MM_DT, isOutput=False
    )
    bv = nc.declare_dram_parameter("bv", [OUT_DIM, 1], F32, isOutput=False)
    ring = min(iters, 8)
    outT = nc.declare_dram_parameter(
        "outT", [OUT_DIM, ring * BSH], F32, isOutput=True
    )

    with tile.TileContext(nc) as tc:
        with (
            tc.tile_pool(name="pool", bufs=1) as pool,
            tc.tile_pool(
                name="psum", bufs=1, space=bass.MemorySpace.PSUM,
            ) as psum,
        ):
            # loop-invariant constants, loaded once per NEFF execution:
            # tap offsets, output bias, w-shift, and the weight bank
            # (weights are pass-invariant, so they stay resident in SBUF)
            ICW = pool.tile([128, cfg.nlanes, BSH], MM_DT, tag="ICW", bufs=1)
            nc.sync.dma_start(_flat(ICW[:]), icw[:])
            BV = pool.tile([OUT_DIM, 1], F32, tag="BV", bufs=1)
            nc.sync.dma_start(BV[:], bv[:])
            WT = pool.tile([128, cfg.ntiles, OUT_DIM], MM_DT, tag="WT", bufs=1)
            nc.sync.dma_start(WT[:].rearrange("p a b -> p (a b)"), wt[:])

            def body():
                pending = []  # (PS, out-slice) awaiting their output stage
                depth = cfg.skew_depth if cfg.skew else 0
                for it in range(iters):
                    r = it % ring
                    o = outT[:, r * BSH : (r + 1) * BSH]
                    if cfg.out_first and len(pending) >= depth and pending:
                        _emit_out(nc, pool, *pending.pop(0), BV, cfg)
                    PS = _emit_iter(nc, pool, psum, xs, WT, ICW, cfg)
                    pending.append((PS, o))
                    if not cfg.out_first and len(pending) > depth:
                        _emit_out(nc, pool, *pending.pop(0), BV, cfg)
                for p in pending:
                    _emit_out(nc, pool, *p, BV, cfg)

            if loop_n > 1:
                with tc.For_i(0, loop_n, 1):
                    body()
            else:
                body()

    return nc


def _prune_dominated_waits(nc):
    """Drop semaphore waits provably satisfied by an earlier wait on the
    same engine queue: sequencers process waits in queue order and the
    tile framework's semaphores count up monotonically within a block
    (loop bodies reset at the iteration barrier), so a later sem-ge wait
    on the same (engine, semaphore) with an equal or lower threshold is
    redundant.  Each pruned wait removes one legalize-NoOp."""
    for blk in nc.m.functions[0].blocks:
        seen = {}  # (engine, sem id) -> max threshold already waited
        for ins in blk.instructions:
            si = ins.sync_info
            if si is None or not si.on_wait:
                continue
            kept = []
            for w in si.on_wait:
                if (w.sync_type == "semaphore" and w.wait_mode == "sem-ge-imm"
                        and w.wait_reg is None):
                    key = (ins.engine, w.id)
                    if seen.get(key, -1) >= w.wait_value:
                        continue  # dominated: drop
                    seen[key] = w.wait_value
                kept.append(w)
            if len(kept) != len(si.on_wait):
                ins.sync_info = mybir.SyncInfo(
                    on_wait=kept, on_update=list(si.on_update)
                )
    return nc


def _legalize_waits(nc):
    """Walrus codegen allows only one semaphore wait per compute/DMA
    instruction; move extra waits onto inserted same-engine NoOps."""
    # NOTE: _prune_dominated_waits is intentionally NOT applied: pruning
    # the 5 theoretically-dominated waits hung the device (the domination
    # rule does not hold for at least one semaphore class here).
    for blk in nc.m.functions[0].blocks:
        out = []
        for ins in blk.instructions:
            si = ins.sync_info
            if si is not None and len(si.on_wait) > 1:
                waits = list(si.on_wait)
                for i, w in enumerate(waits[:-1]):
                    nop = mybir.InstNoOp(
                        name=f"{ins.name}-lw{i}", engine=ins.engine, ins=[], outs=[]
                    )
                    nop.sync_info = mybir.SyncInfo(on_wait=[w], on_update=[])
                    out.append(nop)
                ins.sync_info = mybir.SyncInfo(
                    on_wait=[waits[-1]], on_update=list(si.on_update)
                )
            out.append(ins)
        blk.instructions = out
    return nc


def prepare_inputs(x, grid, coef, scale_base, scale_sp, mask):
    x = np.ascontiguousarray(x, dtype=np.float32)
    grid = np.asarray(grid, dtype=np.float32)
    coef = np.asarray(coef, dtype=np.float64)
    g = grid[0].astype(np.float64)
    h = (g[-1] - g[0]) / (len(g) - 1)
    g0ext = g[0] - KDEG * h
    inv_h = 1.0 / h
    bias_v = -g0ext * inv_h

    vmin = float(x.min()) * inv_h + bias_v
    vmax = float(x.max()) * inv_h + bias_v
    cfg = Cfg(inv_h, bias_v, vmin, vmax)

    import ml_dtypes
    from math import comb

    bfq = lambda a: np.asarray(a, np.float32).astype(ml_dtypes.bfloat16)

    # fold Delta^4 (and the 1/6) into per-tap weights: W[s,n]
    W = np.zeros((SIZE, NT))
    for j in range(NB):
        for m in range(KDEG + 2):
            W[:, j + m] += coef[:, j] / 6.0 * ((-1) ** m) * comb(KDEG + 1, m)
    # cubic-polynomial fold of taps n <= msp, rebased at cc
    a = np.zeros((SIZE, 4))
    for n in range(0, cfg.msp + 1):
        t = cfg.cc - n
        a[:, 0] += W[:, n] * t**3
        a[:, 1] += W[:, n] * 3 * t**2
        a[:, 2] += W[:, n] * 3 * t
        a[:, 3] += W[:, n]

    sbm = np.asarray(scale_base, np.float64) * np.asarray(mask, np.float64)
    sspm = np.asarray(scale_sp, np.float64) * np.asarray(mask, np.float64)
    # 1/h^k folds for the x-space lane bank; mirrored knots get the
    # relu(knot-x)^3 = -min(u^3,0) sign fold
    rows = [sbm, sspm * a[:, 1] * inv_h, sspm * a[:, 2] * inv_h**2,
            sspm * a[:, 3] * inv_h**3]
    rows += [
        sspm * W[:, n] * inv_h**3
        * (-1.0 if (n <= cfg.msp and not cfg.dd2) else 1.0)
        for n in cfg.live
    ]
    wt = np.empty((cfg.ntiles * 128, OUT_DIM), np.float32)
    for k, r in enumerate(rows):
        wt[k * 128 : (k + 1) * 128] = r.reshape(OUT_DIM, IN_DIM).T
    # [k*i, o] -> [i, k*o] so each partition's weights are contiguous
    wt = np.ascontiguousarray(
        wt.reshape(cfg.ntiles, IN_DIM, OUT_DIM).transpose(1, 0, 2).reshape(
            IN_DIM, cfg.ntiles * OUT_DIM
        )
    ).astype(mybir.dt.np(MM_DT))

    # per-o output bias: constant poly term summed over i
    bv = np.ascontiguousarray(
        (sspm * a[:, 0]).reshape(OUT_DIM, IN_DIM).sum(axis=1)[:, None],
        dtype=np.float32,
    )
    # lane offsets: knot x-positions, then the poly-center lane
    offs = bfq([g0ext + n * h for n in cfg.live] + [g0ext + cfg.cc * h])
    icw = np.ascontiguousarray(
        np.broadcast_to(
            np.repeat(offs, BSH)[None, :], (128, cfg.nlanes * BSH)
        )
    )

    xT = np.ascontiguousarray(x.T).astype(mybir.dt.np(MM_DT))  # [i, b] bf16
    in_maps = [
        {
            "xs": np.ascontiguousarray(xT[:, c * BSH : (c + 1) * BSH]),
            "wt": wt,
            "icw": icw,
            "bv": bv,
        }
        for c in range(N_CORES)
    ]
    return in_maps, cfg


class Runner:
    """AOT-compiled fast-dispatch executor for a Bass program on N cores.

    Compiles once (jit trace + NEFF build happen here, not per call);
    subsequent __call__s hit JAX's C++ fast path - per-call cost is the
    axon dispatch plus device execution only.
    """

    def __init__(self, nc, n_cores: int = N_CORES):
        import jax
        from jax.sharding import Mesh, NamedSharding, PartitionSpec

        from concourse import bass2jax
        from concourse.bass2jax import (
            _bass_exec_p,
            fast_dispatch_compile,
            install_neuronx_cc_hook,
        )

        try:
            from jax.experimental.shard_map import shard_map
        except ImportError:  # newer jax
            from jax import shard_map

        install_neuronx_cc_hook()
        self.jax = jax
        self.n_cores = n_cores
        part_name = nc.partition_id_tensor.name if nc.partition_id_tensor else None
        assert nc.dbg_addr is None

        in_names, in_shapes, out_names, out_avals = [], [], [], []
        for alloc in nc.m.functions[0].allocations:
            if not isinstance(alloc, mybir.MemoryLocationSet):
                continue
            name = alloc.memorylocations[0].name
            if alloc.kind == "ExternalInput":
                if name != part_name:
                    in_names.append(name)
                    in_shapes.append(
                        (tuple(alloc.tensor_shape), mybir.dt.np(alloc.dtype))
                    )
            elif alloc.kind == "ExternalOutput":
                out_names.append(name)
                out_avals.append(
                    jax.core.ShapedArray(
                        tuple(alloc.tensor_shape), mybir.dt.np(alloc.dtype)
                    )
                )
        self.in_names = in_names
        self.out_names = out_names
        self.out_avals = out_avals
        # The kernel writes every element of its outputs, so they are not
        # passed as (donated zero) operands - results are fresh buffers.
        all_in_names = list(in_names)
        if part_name is not None:
            all_in_names.append(part_name)

        def _body(*args):
            operands = list(args)
            if part_name is not None:
                operands.append(bass2jax.partition_id_tensor())
            outs = _bass_exec_p.bind(
                *operands,
                out_avals=tuple(out_avals),
                in_names=tuple(all_in_names),
                out_names=tuple(out_names),
                lowering_input_output_aliases=(),
                sim_require_finite=True,
                sim_require_nnan=True,
                nc=nc,
            )
            return tuple(outs)

        devices = jax.devices()[:n_cores]
        self.mesh = Mesh(np.asarray(devices), ("core",))
        self.sharding = NamedSharding(self.mesh, PartitionSpec("core"))
        in_specs = (PartitionSpec("core"),) * len(in_names)
        out_specs = (PartitionSpec("core"),) * len(out_names)
        jitted = jax.jit(
            shard_map(
                _body,
                mesh=self.mesh,
                in_specs=in_specs,
                out_specs=out_specs,
                check_rep=False,
            ),
            keep_unused=True,
        )

        def compile_fn():
            abstract = [
                jax.ShapeDtypeStruct((n_cores * s[0], *s[1:]), d)
                for (s, d) in in_shapes
            ]
            return jitted.lower(*abstract).compile()

        self.compiled = fast_dispatch_compile(compile_fn)

    def stage(self, in_maps):
        """Concat per-core inputs on axis 0 and put on device (committed)."""
        concat = [
            np.concatenate(
                [np.asarray(in_maps[c][nm]) for c in range(self.n_cores)], axis=0
            )
            for nm in self.in_names
        ]
        args = [self.jax.device_put(a, self.sharding) for a in concat]
        self.jax.block_until_ready(args)
        return args

    def __call__(self, args):
        return self.compiled(*args)

    def fetch_np(self, outs):
        """outs -> list of per-core np arrays for output 0."""
        arr = np.asarray(outs[0])
        s = self.out_avals[0].shape
        return arr.reshape(self.n_cores, *s)


def _assemble(per_core_outT):
    """per-core outT [OUT_DIM, BSH] -> full [BATCH, OUT_DIM]."""
    return np.ascontiguousarray(
        np.concatenate([o.T for o in per_core_outT], axis=0), dtype=np.float32
    )


def run(inputs: dict, trace: bool = False, **spmd_kwargs):
    """Stock-path execution (kept for debugging / fallback)."""
    from concourse.bass_utils import run_bass_kernel_spmd

    in_maps, cfg = prepare_inputs(**inputs)
    nc = _legalize_waits(build_program(cfg))
    res = run_bass_kernel_spmd(
        nc, in_maps, list(range(N_CORES)), trace=trace, **spmd_kwargs
    )
    out = _assemble([np.asarray(res.results[c]["outT"]) for c in range(N_CORES)])
    return out, res


def kernel(**inputs) -> np.ndarray:
    assert inputs["x"].shape == (BATCH, IN_DIM)
    in_maps, cfg = prepare_inputs(**inputs)
    nc = _legalize_waits(build_program(cfg))
    try:
        runner = Runner(nc)
        outs = runner(runner.stage(in_maps))
        return _assemble(list(runner.fetch_np(outs)))
    except Exception:
        from concourse.bass_utils import run_bass_kernel_spmd

        res = run_bass_kernel_spmd(nc, in_maps, list(range(N_CORES)))
        return _assemble(
            [np.asarray(res.results[c]["outT"]) for c in range(N_CORES)]
        )


# revision 58
# speedup vs baseline: 1.0134x; 1.0134x over previous
"""KANLayer (in=128, out=128, num=5, k=3, batch=1024) on 8 trn2 NeuronCores.

Math: out[b,o] = sum_i mask*scale_base*silu(x[b,i])
              + sum_i mask*scale_sp*sum_j coef[(o,i),j]*B_j(x[b,i])
The reference grid is a uniform linspace broadcast to all rows, so the
Cox-de-Boor bases are cardinal cubic B-splines, B_j(v) = Delta^4
relu(v-n)^3/6 at n=j with v = (x - g0ext)/h.  The Delta^4 is a fixed
linear map from truncated-power taps to bases, so it is folded into the
spline weights host-side: y_sp = sum_n W[s,n]*relu(v-n)^3 with
W = (coef/6) @ Delta4^T.  Three exact host-side rewrites then keep every
on-device lane value small enough for bf16 throughout:

  * taps with knot >= max(v) are identically zero -> dropped;
  * taps with knot <= min(v) satisfy relu(v-n)^3 = (v-n)^3 -> folded
    into a cubic polynomial;
  * live taps below the range midpoint use the mirror identity
    relu(v-n)^3 = (v-n)^3 + relu(n-v)^3, the cubic again folded into
    the polynomial.

Everything is evaluated in x-space (u = x - knot_x, the 1/h^k scales
folded into the weights).  The device-side work per pass is one bf16
lane bank [128, 7, 128] = {6 live knots, 1 poly-center lane} built by
four DVE ops (subtract; cube via square+multiply with the square on the
scalar engine; relu as min over mirrored lanes + max over direct lanes,
with the mirror sign folded into the weights since relu(knot-x)^3 =
-min(u^3, 0) and relu/cube commute), plus silu on the scalar engine.
The poly-center lane's powers in DD/U2/U3 are the polynomial features
w, w^2, w^3 for free, and the constant term rides the PSUM->SBUF output
copy as a per-partition bias.  The contraction is 10 accumulated
128x128x128 bf16 PE matmuls per core against a weight bank that stays
resident in SBUF.  All elementwise ops run in bf16 (2x/4x DVE modes).

Sharding: batch 1024 -> 128 per core (independent; no collectives).

Execution: the Bass program is AOT-compiled once into a PJRT executable
(fast-dispatch, no per-call retrace) and dispatched on cores 0-7; falls
back to the stock run_bass_kernel_spmd path on any failure.
"""

import numpy as np

import concourse.bass as bass
import concourse.mybir as mybir
import concourse.tile as tile

AF = mybir.ActivationFunctionType
ALU = mybir.AluOpType
F32 = mybir.dt.float32
BF16 = mybir.dt.bfloat16

N_CORES = 8
BATCH = 1024
IN_DIM = 128
OUT_DIM = 128
NUM, KDEG = 5, 3
NB = NUM + KDEG          # 8 basis functions
NT = NB + KDEG + 1       # 12 truncated-power taps
BSH = BATCH // N_CORES   # 128 batch elems per core
SIZE = IN_DIM * OUT_DIM

MM_DT = BF16  # matmul operand dtype


def _bcast_mid(ap2d, n):
    """[128, F] AP -> [128, n, F] with zero-stride middle dim."""
    p, f = ap2d.shape
    return ap2d.rearrange("p (a b) -> p a b", a=1).broadcast_to([p, n, f])


def _flat(ap3d):
    """[128, a, b] AP -> [128, a*b]."""
    return ap3d.rearrange("p a b -> p (a b)")


class Cfg:
    """Data-dependent program constants (live taps, split, poly center)."""

    def __init__(self, inv_h, bias_v, vmin, vmax):
        self.inv_h = float(inv_h)
        self.bias_v = float(bias_v)
        nlo = int(np.floor(vmin))          # taps <= nlo: always-on cubic
        nhi = int(np.ceil(vmax))           # taps >= nhi: identically zero
        self.cc = float((vmin + vmax) / 2)  # poly/tap rebase center
        self.msp = int(np.floor(self.cc))   # mirror split knot
        self.live = [n for n in range(max(nlo + 1, 0), min(nhi, NT))]
        self.nmir = sum(1 for n in self.live if n <= self.msp)
        self.ntap = len(self.live)
        self.nlanes = self.ntap + 1        # taps + the w-lane (t=0)
        self.ntiles = 4 + self.ntap        # silu, w, w^2, w^3, taps
        self.h = 1.0 / self.inv_h
        self.g0ext = -self.bias_v * self.h  # extended-grid origin (x-space)
        self.cx = self.g0ext + self.cc * self.h  # poly center, x-space
        # engine assignment knobs (chosen by timeline-sim + on-device sweep)
        # (GPSIMD cannot access PSUM, so o_eng is scalar or vector only)
        self.o_eng = "split"   # PSUM->SBUF output copy engine
        self.u2_act = True     # lane square on Act engine (else DVE)
        self.nbufs = 3         # passes in flight (tile buffer depth)
        self.out_pool = False  # issue the output DMA from the Pool queue
        self.skew = True       # emit pass k's output stage after pass k+1
        self.skew_depth = 1    # passes between compute and its output stage
        self.psum_extra = 1    # extra PSUM buffers beyond nbufs
        self.out_first = False  # emit pending output stage before compute
        self.out_batch = 2     # passes per output DMA (1 or 2)
        # GpSimd per-op overhead is ~1us on real HW (cost model is wrong
        # about it) - never assign per-pass ops there
        self.poly_pool = False  # poly-feature lane pipeline on GpSimd
        self.o_dve_cols = 32   # columns of the output copy done on DVE
        self.dd2 = False       # two-op DD (mir lanes negated) + single max


def _emit_out(nc, pool, items, o_dram, BV, cfg):
    """Output stage for out_batch passes: PSUM -> SBUF copies (+ poly-
    constant bias), then one DMA covering all of them (fewer DMA
    instructions and fewer write-after-read waits on the copy engines)."""
    nb = len(items)
    O = pool.tile([OUT_DIM, nb, BSH], F32, tag="O", bufs=cfg.nbufs)
    for j, PS in enumerate(items):
        Oj = O[:, j, :]
        if cfg.o_eng == "vector":
            nc.vector.tensor_scalar(Oj[:], PS[:], BV[:, 0:1], None, ALU.add)
        elif cfg.o_eng == "split":  # split the copy across DVE and Act
            h = cfg.o_dve_cols
            nc.vector.tensor_scalar(Oj[:, :h], PS[:, :h], BV[:, 0:1], None,
                                    ALU.add)
            nc.scalar.activation(Oj[:, h:], PS[:, h:], AF.Identity,
                                 bias=BV[:, 0:1])
        else:
            nc.scalar.activation(Oj[:], PS[:], AF.Identity, bias=BV[:, 0:1])
    (nc.gpsimd if cfg.out_pool else nc.sync).dma_start(
        o_dram[:, :], _flat(O[:])
    )


def _emit_iter(nc, pool, psum, xs, WT, ICW, cfg):
    """One pass's compute: load, lane bank, 4+ntap matmuls -> PSUM tile."""
    ntap, nmir = cfg.ntap, cfg.nmir
    ib = cfg.nbufs  # intermediate-tile buffering (passes in flight)
    # multi-buffered input load (x arrives pre-cast to bf16): later
    # passes' DMAs issue while this pass computes (standard prefetch)
    XB = pool.tile([128, BSH], MM_DT, tag="XB", bufs=ib)
    nc.sync.dma_start(XB[:], xs[:])

    S = pool.tile([128, BSH], MM_DT, tag="S", bufs=ib)  # silu(x), K-tile 0
    nc.scalar.activation(S[:], XB[:], AF.Silu)

    # lane bank in x-space: u = x - knot_x per live knot, plus a final
    # lane at the poly center whose powers are the poly features (the
    # 1/h^k scalings are folded into the weights host-side).  relu(u)^3 =
    # relu(u^3), and the mirrored (below-split) knots need relu(knot-x)^3
    # = -min(u^3, 0), so the sign fold goes into their weights and the
    # relu stage is one min over mirrored lanes + one max over direct
    # lanes (4x-mode tensor_scalar).  All bf16 (2x/4x DVE modes).
    # with poly_pool the bank holds only the tap lanes; the poly-center
    # lane (no relu needed) runs as its own 3-op pipeline on GpSimd
    nb = ntap if cfg.poly_pool else cfg.nlanes
    DD = pool.tile([128, nb, BSH], MM_DT, tag="DD", bufs=ib)
    if cfg.dd2:
        # mirrored lanes hold knot-x directly, so the relu stage is one
        # max over all tap lanes and no weight sign fold is needed
        nc.vector.tensor_tensor(
            DD[:, :nmir, :], ICW[:, :nmir, :], _bcast_mid(XB[:], nmir),
            ALU.subtract,
        )
        nc.vector.tensor_tensor(
            DD[:, nmir:, :], _bcast_mid(XB[:], nb - nmir), ICW[:, nmir:nb, :],
            ALU.subtract,
        )
    else:
        nc.vector.tensor_tensor(
            DD[:], _bcast_mid(XB[:], nb), ICW[:, :nb, :], ALU.subtract
        )
    U2 = pool.tile([128, nb, BSH], MM_DT, tag="U2", bufs=ib)
    if cfg.u2_act:
        nc.scalar.activation(_flat(U2[:]), _flat(DD[:]), AF.Square)
    else:
        nc.vector.tensor_tensor(_flat(U2[:]), _flat(DD[:]), _flat(DD[:]),
                                ALU.mult)
    U3 = pool.tile([128, nb, BSH], MM_DT, tag="U3", bufs=ib)
    nc.vector.tensor_tensor(_flat(U3[:]), _flat(U2[:]), _flat(DD[:]), ALU.mult)
    R3 = pool.tile([128, ntap, BSH], MM_DT, tag="R3", bufs=ib)
    if cfg.dd2:
        nc.vector.tensor_scalar(
            _flat(R3[:]), _flat(U3[:])[:, : ntap * BSH], 0.0, None, ALU.max
        )
    else:
        nc.vector.tensor_scalar(
            _flat(R3[:])[:, : nmir * BSH], _flat(U3[:])[:, : nmir * BSH],
            0.0, None, ALU.min,
        )
        nc.vector.tensor_scalar(
            _flat(R3[:])[:, nmir * BSH :],
            _flat(U3[:])[:, nmir * BSH : ntap * BSH], 0.0, None, ALU.max,
        )
    if cfg.poly_pool:
        DP = pool.tile([128, BSH], MM_DT, tag="DP", bufs=ib)
        nc.gpsimd.tensor_scalar(DP[:], XB[:], cfg.cx, None, ALU.subtract)
        P2 = pool.tile([128, BSH], MM_DT, tag="P2", bufs=ib)
        nc.gpsimd.tensor_tensor(P2[:], DP[:], DP[:], ALU.mult)
        P3 = pool.tile([128, BSH], MM_DT, tag="P3", bufs=ib)
        nc.gpsimd.tensor_tensor(P3[:], P2[:], DP[:], ALU.mult)
        poly = [DP[:], P2[:], P3[:]]
    else:
        poly = [DD[:, ntap, :], U2[:, ntap, :], U3[:, ntap, :]]

    # out^T[o,b] = sum_k WT[:,k,:]^T @ rhs_k, K = ntiles*128
    PS = psum.tile([OUT_DIM, BSH], F32, tag="PS",
                   bufs=cfg.nbufs + (cfg.psum_extra if cfg.skew else 0))
    rhss = [S[:]] + poly
    rhss += [R3[:, t, :] for t in range(ntap)]
    for k, rhs in enumerate(rhss):
        nc.tensor.matmul(
            PS[:], WT[:, k, :], rhs, start=(k == 0), stop=(k == len(rhss) - 1)
        )
    return PS


def build_program(
    cfg, iters: int = 1, pipelined: bool = False, loop_n: int = 1
):
    """One SPMD NeuronCore program; per-core inputs differ only in data.

    iters > 1 unrolls the full kernel back-to-back inside one NEFF, and
    loop_n > 1 wraps the unrolled body in a hardware For_i loop (total
    passes = iters * loop_n) - used to measure per-iteration HW execution
    time without a profiler while keeping the NEFF small.

    Successive passes write a small ring of output slices (a real stream
    writes each batch's result to a distinct buffer; reusing one address
    would add an artificial DRAM write-after-write serialization to the
    measurement).  Slice 0 always holds a complete pass result.
    """
    del pipelined  # legacy knob, superseded by the output ring
    nc = bass.Bass()
    xs = nc.declare_dram_parameter("xs", [IN_DIM, BSH], MM_DT, isOutput=False)
    # weights pre-transposed host-side to [i, k*o] so the load is one
    # contiguous-per-partition DMA
    wt = nc.declare_dram_parameter(
        "wt", [128, cfg.ntiles * OUT_DIM], MM_DT, isOutput=False
    )
    icw = nc.declare_dram_parameter(
        "icw", [128, cfg.nlanes * BSH], MM_DT, isOutput=False
    )
    bv = nc.declare_dram_parameter("bv", [OUT_DIM, 1], F32, isOutput=False)
    ring = min(iters, 8)
    outT = nc.declare_dram_parameter(
        "outT", [OUT_DIM, ring * BSH], F32, isOutput=True
    )

    with tile.TileContext(nc) as tc:
        with (
            tc.tile_pool(name="pool", bufs=1) as pool,
            tc.tile_pool(
                name="psum", bufs=1, space=bass.MemorySpace.PSUM,
            ) as psum,
        ):
            # loop-invariant constants, loaded once per NEFF execution:
            # tap offsets, output bias, w-shift, and the weight bank
            # (weights are pass-invariant, so they stay resident in SBUF)
            ICW = pool.tile([128, cfg.nlanes, BSH], MM_DT, tag="ICW", bufs=1)
            nc.sync.dma_start(_flat(ICW[:]), icw[:])
            BV = pool.tile([OUT_DIM, 1], F32, tag="BV", bufs=1)
            nc.sync.dma_start(BV[:], bv[:])
            WT = pool.tile([128, cfg.ntiles, OUT_DIM], MM_DT, tag="WT", bufs=1)
            nc.sync.dma_start(WT[:].rearrange("p a b -> p (a b)"), wt[:])

            def body():
                pending = []  # (PS, pass index) awaiting their output stage
                depth = cfg.skew_depth if cfg.skew else 0
                ob = min(cfg.out_batch, iters)
                assert iters % ob == 0 and ring % ob == 0

                def flush():
                    grp = [pending.pop(0) for _ in range(ob)]
                    r0 = grp[0][1] % ring
                    _emit_out(nc, pool, [g[0] for g in grp],
                              outT[:, r0 * BSH : (r0 + ob) * BSH], BV, cfg)

                for it in range(iters):
                    PS = _emit_iter(nc, pool, psum, xs, WT, ICW, cfg)
                    pending.append((PS, it))
                    if len(pending) >= depth + ob:
                        flush()
                while pending:
                    flush()

            if loop_n > 1:
                with tc.For_i(0, loop_n, 1):
                    body()
            else:
                body()

    return nc


def _prune_dominated_waits(nc):
    """Drop semaphore waits provably satisfied by an earlier wait on the
    same engine queue: sequencers process waits in queue order and the
    tile framework's semaphores count up monotonically within a block
    (loop bodies reset at the iteration barrier), so a later sem-ge wait
    on the same (engine, semaphore) with an equal or lower threshold is
    redundant.  Each pruned wait removes one legalize-NoOp."""
    for blk in nc.m.functions[0].blocks:
        seen = {}  # (engine, sem id) -> max threshold already waited
        for ins in blk.instructions:
            si = ins.sync_info
            if si is None or not si.on_wait:
                continue
            kept = []
            for w in si.on_wait:
                if (w.sync_type == "semaphore" and w.wait_mode == "sem-ge-imm"
                        and w.wait_reg is None):
                    key = (ins.engine, w.id)
                    if seen.get(key, -1) >= w.wait_value:
                        continue  # dominated: drop
                    seen[key] = w.wait_value
                kept.append(w)
            if len(kept) != len(si.on_wait):
                ins.sync_info = mybir.SyncInfo(
                    on_wait=kept, on_update=list(si.on_update)
                )
    return nc


def _legalize_waits(nc):
    """Walrus codegen allows only one semaphore wait per compute/DMA
    instruction; move extra waits onto inserted same-engine NoOps."""
    # NOTE: _prune_dominated_waits is intentionally NOT applied: pruning
    # the 5 theoretically-dominated waits hung the device (the domination
    # rule does not hold for at least one semaphore class here).
    for blk in nc.m.functions[0].blocks:
        out = []
        for ins in blk.instructions:
            si = ins.sync_info
            if si is not None and len(si.on_wait) > 1:
                waits = list(si.on_wait)
                for i, w in enumerate(waits[:-1]):
                    nop = mybir.InstNoOp(
                        name=f"{ins.name}-lw{i}", engine=ins.engine, ins=[], outs=[]
                    )
                    nop.sync_info = mybir.SyncInfo(on_wait=[w], on_update=[])
                    out.append(nop)
                ins.sync_info = mybir.SyncInfo(
                    on_wait=[waits[-1]], on_update=list(si.on_update)
                )
            out.append(ins)
        blk.instructions = out
    return nc


def prepare_inputs(x, grid, coef, scale_base, scale_sp, mask):
    x = np.ascontiguousarray(x, dtype=np.float32)
    grid = np.asarray(grid, dtype=np.float32)
    coef = np.asarray(coef, dtype=np.float64)
    g = grid[0].astype(np.float64)
    h = (g[-1] - g[0]) / (len(g) - 1)
    g0ext = g[0] - KDEG * h
    inv_h = 1.0 / h
    bias_v = -g0ext * inv_h

    vmin = float(x.min()) * inv_h + bias_v
    vmax = float(x.max()) * inv_h + bias_v
    cfg = Cfg(inv_h, bias_v, vmin, vmax)

    import ml_dtypes
    from math import comb

    bfq = lambda a: np.asarray(a, np.float32).astype(ml_dtypes.bfloat16)

    # fold Delta^4 (and the 1/6) into per-tap weights: W[s,n]
    W = np.zeros((SIZE, NT))
    for j in range(NB):
        for m in range(KDEG + 2):
            W[:, j + m] += coef[:, j] / 6.0 * ((-1) ** m) * comb(KDEG + 1, m)
    # cubic-polynomial fold of taps n <= msp, rebased at cc
    a = np.zeros((SIZE, 4))
    for n in range(0, cfg.msp + 1):
        t = cfg.cc - n
        a[:, 0] += W[:, n] * t**3
        a[:, 1] += W[:, n] * 3 * t**2
        a[:, 2] += W[:, n] * 3 * t
        a[:, 3] += W[:, n]

    sbm = np.asarray(scale_base, np.float64) * np.asarray(mask, np.float64)
    sspm = np.asarray(scale_sp, np.float64) * np.asarray(mask, np.float64)
    # 1/h^k folds for the x-space lane bank; mirrored knots get the
    # relu(knot-x)^3 = -min(u^3,0) sign fold
    rows = [sbm, sspm * a[:, 1] * inv_h, sspm * a[:, 2] * inv_h**2,
            sspm * a[:, 3] * inv_h**3]
    rows += [
        sspm * W[:, n] * inv_h**3
        * (-1.0 if (n <= cfg.msp and not cfg.dd2) else 1.0)
        for n in cfg.live
    ]
    wt = np.empty((cfg.ntiles * 128, OUT_DIM), np.float32)
    for k, r in enumerate(rows):
        wt[k * 128 : (k + 1) * 128] = r.reshape(OUT_DIM, IN_DIM).T
    # [k*i, o] -> [i, k*o] so each partition's weights are contiguous
    wt = np.ascontiguousarray(
        wt.reshape(cfg.ntiles, IN_DIM, OUT_DIM).transpose(1, 0, 2).reshape(
            IN_DIM, cfg.ntiles * OUT_DIM
        )
    ).astype(mybir.dt.np(MM_DT))

    # per-o output bias: constant poly term summed over i
    bv = np.ascontiguousarray(
        (sspm * a[:, 0]).reshape(OUT_DIM, IN_DIM).sum(axis=1)[:, None],
        dtype=np.float32,
    )
    # lane offsets: knot x-positions, then the poly-center lane
    offs = bfq([g0ext + n * h for n in cfg.live] + [g0ext + cfg.cc * h])
    icw = np.ascontiguousarray(
        np.broadcast_to(
            np.repeat(offs, BSH)[None, :], (128, cfg.nlanes * BSH)
        )
    )

    xT = np.ascontiguousarray(x.T).astype(mybir.dt.np(MM_DT))  # [i, b] bf16
    in_maps = [
        {
            "xs": np.ascontiguousarray(xT[:, c * BSH : (c + 1) * BSH]),
            "wt": wt,
            "icw": icw,
            "bv": bv,
        }
        for c in range(N_CORES)
    ]
    return in_maps, cfg


class Runner:
    """AOT-compiled fast-dispatch executor for a Bass program on N cores.

    Compiles once (jit trace + NEFF build happen here, not per call);
    subsequent __call__s hit JAX's C++ fast path - per-call cost is the
    axon dispatch plus device execution only.
    """

    def __init__(self, nc, n_cores: int = N_CORES):
        import jax
        from jax.sharding import Mesh, NamedSharding, PartitionSpec

        from concourse import bass2jax
        from concourse.bass2jax import (
            _bass_exec_p,
            fast_dispatch_compile,
            install_neuronx_cc_hook,
        )

        try:
            from jax.experimental.shard_map import shard_map
        except ImportError:  # newer jax
            from jax import shard_map

        install_neuronx_cc_hook()
        self.jax = jax
        self.n_cores = n_cores
        part_name = nc.partition_id_tensor.name if nc.partition_id_tensor else None
        assert nc.dbg_addr is None

        in_names, in_shapes, out_names, out_avals = [], [], [], []
        for alloc in nc.m.functions[0].allocations:
            if not isinstance(alloc, mybir.MemoryLocationSet):
                continue
            name = alloc.memorylocations[0].name
            if alloc.kind == "ExternalInput":
                if name != part_name:
                    in_names.append(name)
                    in_shapes.append(
                        (tuple(alloc.tensor_shape), mybir.dt.np(alloc.dtype))
                    )
            elif alloc.kind == "ExternalOutput":
                out_names.append(name)
                out_avals.append(
                    jax.core.ShapedArray(
                        tuple(alloc.tensor_shape), mybir.dt.np(alloc.dtype)
                    )
                )
        self.in_names = in_names
        self.out_names = out_names
        self.out_avals = out_avals
        # The kernel writes every element of its outputs, so they are not
        # passed as (donated zero) operands - results are fresh buffers.
        all_in_names = list(in_names)
        if part_name is not None:
            all_in_names.append(part_name)

        def _body(*args):
            operands = list(args)
            if part_name is not None:
                operands.append(bass2jax.partition_id_tensor())
            outs = _bass_exec_p.bind(
                *operands,
                out_avals=tuple(out_avals),
                in_names=tuple(all_in_names),
                out_names=tuple(out_names),
                lowering_input_output_aliases=(),
                sim_require_finite=True,
                sim_require_nnan=True,
                nc=nc,
            )
            return tuple(outs)

        devices = jax.devices()[:n_cores]
        self.mesh = Mesh(np.asarray(devices), ("core",))
        self.sharding = NamedSharding(self.mesh, PartitionSpec("core"))
        in_specs = (PartitionSpec("core"),) * len(in_names)
        out_specs = (PartitionSpec("core"),) * len(out_names)
        jitted = jax.jit(
            shard_map(
                _body,
                mesh=self.mesh,
                in_specs=in_specs,
                out_specs=out_specs,
                check_rep=False,
            ),
            keep_unused=True,
        )

        def compile_fn():
            abstract = [
                jax.ShapeDtypeStruct((n_cores * s[0], *s[1:]), d)
                for (s, d) in in_shapes
            ]
            return jitted.lower(*abstract).compile()

        self.compiled = fast_dispatch_compile(compile_fn)

    def stage(self, in_maps):
        """Concat per-core inputs on axis 0 and put on device (committed)."""
        concat = [
            np.concatenate(
                [np.asarray(in_maps[c][nm]) for c in range(self.n_cores)], axis=0
            )
            for nm in self.in_names
        ]
        args = [self.jax.device_put(a, self.sharding) for a in concat]
        self.jax.block_until_ready(args)
        return args

    def __call__(self, args):
        return self.compiled(*args)

    def fetch_np(self, outs):
        """outs -> list of per-core np arrays for output 0."""
        arr = np.asarray(outs[0])
        s = self.out_avals[0].shape
        return arr.reshape(self.n_cores, *s)


def _assemble(per_core_outT):
    """per-core outT [OUT_DIM, BSH] -> full [BATCH, OUT_DIM]."""
    return np.ascontiguousarray(
        np.concatenate([o.T for o in per_core_outT], axis=0), dtype=np.float32
    )


def run(inputs: dict, trace: bool = False, **spmd_kwargs):
    """Stock-path execution (kept for debugging / fallback)."""
    from concourse.bass_utils import run_bass_kernel_spmd

    in_maps, cfg = prepare_inputs(**inputs)
    nc = _legalize_waits(build_program(cfg))
    res = run_bass_kernel_spmd(
        nc, in_maps, list(range(N_CORES)), trace=trace, **spmd_kwargs
    )
    out = _assemble([np.asarray(res.results[c]["outT"]) for c in range(N_CORES)])
    return out, res


def kernel(**inputs) -> np.ndarray:
    assert inputs["x"].shape == (BATCH, IN_DIM)
    in_maps, cfg = prepare_inputs(**inputs)
    nc = _legalize_waits(build_program(cfg))
    try:
        runner = Runner(nc)
        outs = runner(runner.stage(in_maps))
        return _assemble(list(runner.fetch_np(outs)))
    except Exception:
        from concourse.bass_utils import run_bass_kernel_spmd

        res = run_bass_kernel_spmd(nc, in_maps, list(range(N_CORES)))
        return _assemble(
            [np.asarray(res.results[c]["outT"]) for c in range(N_CORES)]
        )
